# revision 36
# baseline (speedup 1.0000x reference)
"""DeepTreeLSTM Trainium2 Bass kernel.

B=256 perfect binary trees (511 nodes, BFS layout), ChildSum TreeLSTM
bottom-up + MLP head. Data-parallel over trees: 32 trees per NeuronCore
x 8 cores. All device tensors use a transposed "feature-on-partition"
layout: [H (2 chunks of 128 partitions), columns], columns tree-major.

Design (evolved from a 270us bf16 baseline via trace analysis; ScalarE
ACT throughput ~163K gate-elems/partition @1.2GHz is the floor):
  - All gate matmuls fp8e4 (weights x16, dequant folded into the ACT
    scale) with perf_mode=DoubleRow: one PE instruction contracts all
    256 features at ~1.7x the bf16 pair's speed and half the
    instruction count. numpy sim of the fp8 pipeline: rel err 2e-4
    (tolerance 2e-2).
  - Levels processed monolithically (SBUF holds h/c for all levels);
    h stored fp8 (feeds the f/iou matmuls), c stored bf16.
  - Columns stored in per-tree BIT-REVERSED order (X permuted on the
    host): every level's child pairs (2j, 2j+1) become the contiguous
    halves of each tree's block, so all pair sums run as contiguous
    2x-mode DVE adds instead of stride-2 gathers.
  - PSUM: [P,4,512] io tile + a [P,2,512]x2 ring shared by the u and f
    drains (they alternate, so one ring double-buffers both) = 8 banks.
  - ACT instruction grain maximized: sigmoid(i,o) merged into one
    2048-elem drain; tanh(c) at 2048-elem super-tiles.
  - Per-level super-tiles are software-pipelined: the f/ht/cagg pass of
    super st is zipped block-by-block with the deferred iou pass of
    super st-1 so the in-order PE/ACT queues never head-block; deep
    levels (m<=512) split into half-column supers to overlap their
    serial chains.
  - Per-tree h sums come from the ht pair-sum tensors (half the reduce
    reads), chunked per super so no big reduce head-blocks the DVE
    queue.

Contract notes vs the reference: the h input is unused (shape only);
c, b_iou, b_in, b_mid, b_out are all-zero per the problem's input spec,
so the kernel drops them (only U_f_b is a live bias).
"""

import os
import sys

import ml_dtypes
import numpy as np

BFNP = ml_dtypes.bfloat16
E4NP = ml_dtypes.float8_e4m3

for _p in ("/opt/trn_rl_repo", "/root/.axon_site/_ro/trn_rl_repo"):
    if os.path.isdir(_p) and _p not in sys.path:
        sys.path.insert(0, _p)

import concourse.bass as bass
import concourse.mybir as mybir
import concourse.tile as tile
from concourse import bacc
from concourse.bass_utils import run_bass_kernel_spmd

P = 128
F32 = mybir.dt.float32
BF16 = mybir.dt.bfloat16
FP8 = mybir.dt.float8e4
H = 256           # hidden size (2 partition chunks)
NB = 32           # trees per core
LEAF = 256        # leaves per tree
COLS = NB * LEAF  # leaf columns per core = 8192
SUB = 512         # iou subtile (psum-coupled)
SUP = 1024        # elementwise/ACT super-tile
AF = mybir.ActivationFunctionType
OP = mybir.AluOpType
DR = mybir.MatmulPerfMode.DoubleRow
QS = 16.0         # weight quant scale (weights x16, ACT scale 1/16)

_PROG = None


def _build_program():
    nc = bacc.Bacc("TRN2", target_bir_lowering=False, debug=False,
                   num_devices=8)

    xT = nc.dram_tensor("xT", [P, 2, COLS], FP8, kind="ExternalInput")
    wiouT = nc.dram_tensor("wiouT", [P, 2, 768], FP8, kind="ExternalInput")
    uiouT = nc.dram_tensor("uiouT", [P, 2, 768], FP8, kind="ExternalInput")
    ufT = nc.dram_tensor("ufT", [P, 2, 256], FP8, kind="ExternalInput")
    ufb = nc.dram_tensor("ufb", [P, 2], F32, kind="ExternalInput")
    ufb8 = nc.dram_tensor("ufb8", [1, 2, P], FP8, kind="ExternalInput")
    ones8 = nc.dram_tensor("ones8", [1, 512], FP8, kind="ExternalInput")
    winT = nc.dram_tensor("winT", [P, 5, P], BF16, kind="ExternalInput")
    emoT = nc.dram_tensor("emoT", [P, NB], BF16, kind="ExternalInput")
    wmidT = nc.dram_tensor("wmidT", [P, 64], F32, kind="ExternalInput")
    woutT = nc.dram_tensor("woutT", [P, 4], F32, kind="ExternalInput")
    out_t = nc.dram_tensor("out_t", [4, NB], F32, kind="ExternalOutput")

    inv = 1.0 / QS

    with tile.TileContext(nc) as tc:
        with (
            tc.tile_pool(name="wp", bufs=1) as wp,
            tc.tile_pool(name="pers", bufs=1) as pers,
        ):
            wiou_sb = wp.tile([P, 2, 768], FP8)
            uiou_sb = wp.tile([P, 2, 768], FP8)
            uf_sb = wp.tile([P, 2, 256], FP8)
            ufb_sb = wp.tile([P, 2], F32)
            ufb8_sb = wp.tile([1, 2, P], FP8)
            ones_sb = wp.tile([1, 512], FP8)
            win_sb = wp.tile([P, 5, P], BF16)
            emo_sb = wp.tile([P, NB], BF16)
            wmid_sb = wp.tile([P, 64], F32)
            wout_sb = wp.tile([P, 4], F32)
            # leaf-critical weights first; the rest are queued later (after
            # the first leaf X tiles) since they are not needed until L7+
            nc.sync.dma_start(wiou_sb[:], wiouT[:])
            late_dmas = [(uf_sb, ufT), (ufb_sb, ufb), (ufb8_sb, ufb8),
                         (ones_sb, ones8), (uiou_sb, uiouT),
                         (win_sb, winT), (emo_sb, emoT), (wmid_sb, wmidT),
                         (wout_sb, woutT)]

            # persistent level storage: h fp8 (levels 8..1), c bf16
            h_lv = {8: pers.tile([P, 2, COLS], FP8, name="h8")}
            c_lv = {8: pers.tile([P, 2, COLS], BF16, name="c8")}
            for d in range(7, 0, -1):
                m = NB * (2 ** d)
                h_lv[d] = pers.tile([P, 2, m], FP8, name=f"h{d}")
                c_lv[d] = pers.tile([P, 2, m], BF16, name=f"c{d}")
            h_lv[0] = pers.tile([P, 2, NB], BF16, name="h0")
            c_lv[0] = pers.tile([P, 2, NB], BF16, name="c0")
            ht_lv = {d: pers.tile([P, 2, NB * (2 ** d)], FP8, name=f"ht{d}")
                     for d in range(8)}
            hsum = pers.tile([P, 2, NB], F32)
            hlast = pers.tile([P, 2, NB], F32)
            nc.vector.memset(hsum[:], 0.0)
            # tiny dummy sigmoid: pulls ACT_TABLE_LOAD + const loads to t=0,
            # overlapping the input DMAs instead of the first leaf drain
            nc.scalar.activation(hlast[:, :, 0:1], hsum[:, :, 0:1],
                                 AF.Sigmoid)

            with (
                tc.tile_pool(name="pps", bufs=1, space="PSUM") as pps,
                tc.tile_pool(name="pa", bufs=2) as pa,
            ):
                def ka(n):
                    """PE keep-alive: dummy DoubleRow weight loads. The HAM
                    clock gate drops the PE to 1.2GHz after ~3.4us of idle;
                    these occupy the PE during dependency stalls with no
                    semantic effect (every real matmul self-loads)."""
                    for _ in range(n):
                        nc.tensor.ldweights(wiou_sb[:, :, 0:P], perf_mode=DR)

                def iou_sub(rhs, w_sb, w, q, io_sb, u_sb, tag):
                    """One 512-col iou subtile: 6 DoubleRow matmuls into
                    [P,4,512]+[P,2,512] PSUM, then merged sigmoid(i,o)
                    (4w elems) and tanh(u) (2w) drains into super-tile
                    SBUF slices."""
                    psio = pps.tile([P, 4, SUB], F32, tag="psIO", bufs=1,
                                    name=f"psio_{tag}")
                    psu = pps.tile([P, 2, SUB], F32, tag="psUF", bufs=2,
                                   name=f"psu_{tag}")
                    for mm in range(6):
                        dst = psio[:, mm, :w] if mm < 4 else \
                            psu[:, mm - 4, :w]
                        if w >= P:
                            nc.tensor.matmul(dst,
                                             w_sb[:, :, mm * P:(mm + 1) * P],
                                             rhs, start=True, stop=True,
                                             perf_mode=DR)
                        else:
                            for k in range(2):
                                nc.tensor.matmul(
                                    dst, w_sb[:, k, mm * P:(mm + 1) * P],
                                    rhs[:, k, :], start=(k == 0),
                                    stop=(k == 1))
                    qs = slice(q * SUB, q * SUB + w)
                    nc.scalar.activation(io_sb[:, :, qs], psio[:, :, :w],
                                         AF.Sigmoid, scale=inv)
                    nc.scalar.activation(u_sb[:, :, qs], psu[:, :, :w],
                                         AF.Tanh, scale=inv)

                def f_block(jb, wj, c0, h_prev, f_sb, tag):
                    """f gates for one 512-children block: 2 DoubleRow
                    matmuls (one per chunk) into [P,2,512] PSUM, 2 sigmoid
                    drains (per-chunk bias)."""
                    psf = pps.tile([P, 2, SUB], F32, tag="psUF", bufs=2,
                                   name=f"psf_{tag}")
                    cs = slice(c0 + jb * 512, c0 + jb * 512 + wj)
                    for g in range(2):
                        if wj >= P:
                            nc.tensor.matmul(
                                psf[:, g, :wj],
                                uf_sb[:, :, g * P:(g + 1) * P],
                                h_prev[:, :, cs], start=True, stop=True,
                                perf_mode=DR)
                        else:
                            for k in range(2):
                                nc.tensor.matmul(
                                    psf[:, g, :wj],
                                    uf_sb[:, k, g * P:(g + 1) * P],
                                    h_prev[:, k, cs],
                                    start=(k == 0), stop=(k == 1))
                    for g in range(2):
                        nc.scalar.activation(
                            f_sb[:, g, jb * 512:jb * 512 + wj],
                            psf[:, g, :wj],
                            AF.Sigmoid, scale=inv, bias=ufb_sb[:, g:g + 1])

                # ---------------- leaves (level 8) ----------------
                h8, c8 = h_lv[8], c_lv[8]
                for st in range(COLS // SUP):
                    ss = slice(st * SUP, (st + 1) * SUP)
                    io_sb = pa.tile([P, 4, SUP], BF16, tag="iob", bufs=2,
                                    name=f"io_L{st}")
                    u_sb = pa.tile([P, 2, SUP], BF16, tag="ub", bufs=2,
                                   name=f"u_L{st}")
                    for half in range(2):
                        xk = pa.tile([P, 2, 512], FP8, tag="xk", bufs=4,
                                     name=f"xk_{st}_{half}")
                        nc.sync.dma_start(
                            xk[:], xT[:, :, st * SUP + half * 512:
                                      st * SUP + half * 512 + 512])
                        iou_sub(xk[:], wiou_sb, SUB, half, io_sb, u_sb,
                                f"L{st}_{half}")
                    if st == 0:
                        for sb, dr_ in late_dmas:
                            nc.sync.dma_start(sb[:], dr_[:])
                    # c = i*u; h = o*tanh(c)
                    nc.vector.tensor_mul(c8[:, :, ss], io_sb[:, 0:2, :],
                                         u_sb[:])
                    t_sb = pa.tile([P, 2, SUP], BF16, tag="tb", bufs=2,
                                   name=f"tl_{st}")
                    nc.scalar.activation(t_sb[:], c8[:, :, ss], AF.Tanh)
                    nc.vector.tensor_mul(h8[:, :, ss], io_sb[:, 2:4, :],
                                         t_sb[:])
                # last leaf (tree-local 255) of each tree
                nc.vector.tensor_copy(hlast[:], h8[:, :, 255::256])

                # ---------------- levels 7..0 ----------------
                # `pending` carries one deferred iou super-pass ACROSS level
                # boundaries: (emit_q(q), emit_tail(), n_q). With
                # sup_d = min(1024, m/2), children(d, 0) reads only the
                # first half of level d+1, which the carried last super
                # never covers, so the carry is always dependency-safe.
                pending = None

                def flush_pending():
                    nonlocal pending
                    if pending is not None:
                        eq, et, nq, qd = pending
                        for q in range(qd, nq):
                            eq(q)
                        et()
                        pending = None

                for d in range(7, -1, -1):
                    m = NB * (2 ** d)
                    h_prev, c_prev = h_lv[d + 1], c_lv[d + 1]
                    h_cur, c_cur = h_lv[d], c_lv[d]
                    ht = ht_lv[d]
                    sup_d = SUP if m > 512 else max(m // 2, 16)
                    n_tree = 2 ** d          # cols per tree at this level
                    deep = m <= 512

                    def iou_q(args, q, d=d, ht=ht, sup_d=sup_d):
                        (sw, st, io_sb, u_sb, cagg, tag) = args
                        w = min(SUB, sw - q * SUB)
                        base = st * sup_d + q * SUB
                        iou_sub(ht[:, :, base:base + w], uiou_sb, w, q,
                                io_sb, u_sb, f"{tag}_q{q}")

                    def iou_tail(args, d=d, ht=ht, sup_d=sup_d,
                                 n_tree=n_tree, h_cur=h_cur, c_cur=c_cur):
                        (sw, st, io_sb, u_sb, cagg, tag) = args
                        ssl = slice(st * sup_d, st * sup_d + sw)
                        iub = pa.tile([P, 2, SUP], BF16, tag="iub", bufs=2,
                                      name=f"iu_{tag}")
                        nc.vector.tensor_mul(iub[:, :, :sw],
                                             io_sb[:, 0:2, :sw],
                                             u_sb[:, :, :sw])
                        nc.vector.tensor_add(c_cur[:, :, ssl],
                                             iub[:, :, :sw],
                                             cagg[:, :, :sw])
                        t_sb = pa.tile([P, 2, SUP], BF16, tag="tb", bufs=2,
                                       name=f"t_{tag}")
                        nc.scalar.activation(t_sb[:, :, :sw],
                                             c_cur[:, :, ssl], AF.Tanh)
                        nc.vector.tensor_mul(h_cur[:, :, ssl],
                                             io_sb[:, 2:4, :sw],
                                             t_sb[:, :, :sw])
                        # hsum contribution of this super's trees via ht
                        t0 = (st * sup_d) // n_tree
                        tn = sw // n_tree if n_tree <= sw else 1
                        part = pa.tile([P, 2, NB], F32, tag="part", bufs=2,
                                       name=f"part_{tag}")
                        if n_tree > 1:
                            nc.vector.tensor_reduce(
                                part[:, :, :tn],
                                ht[:, :, ssl].rearrange(
                                    "p k (t n) -> p k t n", t=tn),
                                axis=mybir.AxisListType.X, op=OP.add)
                        else:
                            nc.vector.tensor_copy(part[:, :, :tn],
                                                  ht[:, :, ssl])
                        nc.gpsimd.tensor_add(hsum[:, :, t0:t0 + tn],
                                             part[:, :, :tn],
                                             hsum[:, :, t0:t0 + tn])

                    def emit_children(st, sw, cagg, f_sb,
                                      d=d, ht=ht, sup_d=sup_d, deep=deep,
                                      h_prev=h_prev, c_prev=c_prev):
                        """f gates + fc + pair sums for super-tile st,
                        zipped block-by-block with the pending deferred iou
                        pass (keeps the in-order PE queue fed)."""
                        nonlocal pending
                        c0 = 2 * st * sup_d
                        cw = 2 * sw
                        # ht pair sums (iou input) first: they depend only
                        # on h_prev, so they must not queue behind cagg.
                        # bit-reversed column order makes the children pair
                        # (2j, 2j+1) the contiguous halves of each tree's
                        # children block.
                        nd = 2 ** d
                        for k in range(2):
                            hv = h_prev[:, k, c0:c0 + cw].rearrange(
                                "p (t two nd) -> p t two nd", two=2, nd=nd)
                            nc.vector.tensor_add(
                                ht[:, k, st * sup_d:st * sup_d + sw]
                                .rearrange("p (t nd) -> p t nd", nd=nd),
                                hv[:, :, 0, :], hv[:, :, 1, :])
                        nblk = (cw + 511) // 512
                        for jb in range(nblk):
                            wj = min(512, cw - jb * 512)
                            f_block(jb, wj, c0, h_prev, f_sb,
                                    f"B{d}_{st}_{jb}")
                            if pending is not None:
                                eq, et, nq, qd = pending
                                want = min(nq, (jb + 1) * nq // nblk)
                                while qd < want:
                                    eq(qd)
                                    qd += 1
                                pending = (eq, et, nq, qd)
                        csl = slice(c0, c0 + cw)
                        fco = pa.tile([P, 2, 2 * SUP], BF16, tag="fco",
                                      bufs=2, name=f"fco_{d}_{st}")
                        nc.vector.tensor_mul(fco[:, :, :cw],
                                             f_sb[:, :, :cw],
                                             c_prev[:, :, csl])
                        for k in range(2):
                            fv = fco[:, k, :cw].rearrange(
                                "p (t two nd) -> p t two nd", two=2, nd=nd)
                            nc.vector.tensor_add(
                                cagg[:, k, :sw].rearrange(
                                    "p (t nd) -> p t nd", nd=nd),
                                fv[:, :, 0, :], fv[:, :, 1, :])
                        flush_pending()

                    for st in range((m + sup_d - 1) // sup_d):
                        sw = min(sup_d, m - st * sup_d)
                        io_sb = pa.tile([P, 4, SUP], BF16, tag="iob", bufs=2,
                                        name=f"io_{d}_{st}")
                        u_sb = pa.tile([P, 2, SUP], BF16, tag="ub", bufs=2,
                                       name=f"u_{d}_{st}")
                        cagg = pa.tile([P, 2, SUP], BF16, tag="cagg", bufs=2,
                                       name=f"cagg_{d}_{st}")
                        f_sb = pa.tile([P, 2, 2 * SUP], BF16, tag="fb",
                                       bufs=2, name=f"f_{d}_{st}")
                        emit_children(st, sw, cagg, f_sb)
                        args = (sw, st, io_sb, u_sb, cagg, f"B{d}_{st}")
                        nqs = (sw + SUB - 1) // SUB
                        pending = (
                            lambda q, a=args, f=iou_q: f(a, q),
                            lambda a=args, f=iou_tail: f(a),
                            nqs, 0)
                    flush_pending()

                # ---------------- head (fp32 tail) ----------------
                h_root = h_lv[0]
                inner = pa.tile([P, 2, NB], BF16)
                nc.vector.tensor_sub(inner[:], hsum[:], hlast[:])
                nc.vector.tensor_scalar_mul(inner[:], inner[:], 1.0 / 509.0)
                y2_sb = pa.tile([P, NB], F32)
                nc.vector.memset(y2_sb[:], 0.0)

                py1 = pps.tile([P, NB], F32, tag="psUF", bufs=2, name="py1")
                chunks = [h_root[:, 0, :], h_root[:, 1, :],
                          inner[:, 0, :], inner[:, 1, :], emo_sb[:]]
                for k in range(5):
                    nc.tensor.matmul(py1[:], win_sb[:, k, :], chunks[k],
                                     start=(k == 0), stop=(k == 4))
                y1_sb = pa.tile([P, NB], F32)
                nc.scalar.activation(y1_sb[:], py1[:], AF.Relu)
                py2 = pps.tile([64, NB], F32, tag="psUF", bufs=2, name="py2")
                nc.tensor.matmul(py2[:], wmid_sb[:], y1_sb[:])
                nc.scalar.activation(y2_sb[:64, :], py2[:], AF.Relu)
                po = pps.tile([4, NB], F32, tag="psUF", bufs=2, name="po")
                nc.tensor.matmul(po[:], wout_sb[:], y2_sb[:])
                o_sb = pa.tile([4, NB], F32)
                nc.scalar.activation(o_sb[:], po[:], AF.Sigmoid)
                nc.sync.dma_start(out_t[:], o_sb[:])

    nc.finalize()
    return nc


def _chunked(w):
    """[K, M] host array -> [P, K//P, M] device layout (K on partitions)."""
    k, m = w.shape
    return np.ascontiguousarray(w.reshape(k // P, P, m).transpose(1, 0, 2))


def _prep_shared(W_iou, U_iou, b_iou, U_f_w, U_f_b, W_in, b_in, W_mid, b_mid,
                 W_out, b_out):
    f = np.float32
    wiouT = _chunked(np.ascontiguousarray(W_iou.T).astype(f) * QS).astype(E4NP)
    uiouT = _chunked(np.ascontiguousarray(U_iou.T).astype(f) * QS).astype(E4NP)
    ufT = _chunked(np.ascontiguousarray(U_f_w.T).astype(f) * QS).astype(E4NP)
    ufb_h = np.ascontiguousarray(U_f_b.reshape(2, P).T).astype(f)
    ufb8_h = (U_f_b.reshape(1, 2, P).astype(f) * QS).astype(E4NP)
    ones_h = np.ones((1, 512), f).astype(E4NP)
    winT = np.zeros((640, P), f)
    winT[:544] = W_in.T
    winT = _chunked(winT).astype(BFNP)
    wmidT = np.ascontiguousarray(W_mid.T).astype(f)
    woutT = np.zeros((P, 4), f)
    woutT[:64] = W_out.T
    return dict(wiouT=wiouT, uiouT=uiouT, ufT=ufT, ufb=ufb_h,
                ufb8=ufb8_h, ones8=ones_h,
                winT=winT, wmidT=wmidT, woutT=woutT)


def _run(X, emo, shared, trace=False):
    global _PROG
    if _PROG is None:
        _PROG = _build_program()
    nc = _PROG

    # per-tree bit-reversed leaf order: makes every level's child pairs
    # the contiguous halves of each tree's block on device
    br = np.zeros(LEAF, np.int64)
    for i in range(LEAF):
        br[i] = int(format(i, "08b")[::-1], 2)

    in_maps = []
    for cc in range(8):
        Xc = X[cc * NB:(cc + 1) * NB, 255:511, :][:, br, :]
        xT = Xc.transpose(2, 0, 1).reshape(256, COLS)
        xT = np.ascontiguousarray(
            xT.reshape(2, P, COLS).transpose(1, 0, 2)).astype(E4NP)
        emoT = np.zeros((P, NB), BFNP)
        emoT[:32] = emo[cc * NB:(cc + 1) * NB].T.astype(BFNP)
        in_maps.append(dict(xT=xT, emoT=emoT, **shared))

    res = None
    for attempt in range(3):
        try:
            res = run_bass_kernel_spmd(nc, in_maps, core_ids=list(range(8)),
                                       trace=trace)
            break
        except Exception:
            if attempt == 2:
                raise
    out = np.concatenate([res.results[cc]["out_t"].T for cc in range(8)],
                         axis=0)
    return np.ascontiguousarray(out.astype(np.float32)), res


def kernel(X, h, c, emo, W_iou, U_iou, b_iou, U_f_w, U_f_b,
           W_in, b_in, W_mid, b_mid, W_out, b_out, **kwargs):
    X = np.asarray(X, np.float32)
    emo = np.asarray(emo, np.float32)
    shared = _prep_shared(np.asarray(W_iou), np.asarray(U_iou),
                          np.asarray(b_iou), np.asarray(U_f_w),
                          np.asarray(U_f_b), np.asarray(W_in),
                          np.asarray(b_in), np.asarray(W_mid),
                          np.asarray(b_mid), np.asarray(W_out),
                          np.asarray(b_out))
    out, _ = _run(X, emo, shared)
    return out


# revision 37
# speedup vs baseline: 1.0082x; 1.0082x over previous
"""DeepTreeLSTM Trainium2 Bass kernel.

B=256 perfect binary trees (511 nodes, BFS layout), ChildSum TreeLSTM
bottom-up + MLP head. Data-parallel over trees: 32 trees per NeuronCore
x 8 cores. All device tensors use a transposed "feature-on-partition"
layout: [H (2 chunks of 128 partitions), columns], columns tree-major.

Design (evolved from a 270us bf16 baseline via trace analysis; ScalarE
ACT throughput ~163K gate-elems/partition @1.2GHz is the floor):
  - All gate matmuls fp8e4 (weights x16, dequant folded into the ACT
    scale) with perf_mode=DoubleRow: one PE instruction contracts all
    256 features at ~1.7x the bf16 pair's speed and half the
    instruction count. numpy sim of the fp8 pipeline: rel err 2e-4
    (tolerance 2e-2).
  - Levels processed monolithically (SBUF holds h/c for all levels);
    h stored fp8 (feeds the f/iou matmuls), c stored bf16.
  - Columns stored in per-tree BIT-REVERSED order (X permuted on the
    host): every level's child pairs (2j, 2j+1) become the contiguous
    halves of each tree's block, so all pair sums run as contiguous
    2x-mode DVE adds instead of stride-2 gathers.
  - PSUM: [P,4,512] io tile + a [P,2,512]x2 ring shared by the u and f
    drains (they alternate, so one ring double-buffers both) = 8 banks.
  - ACT instruction grain maximized: sigmoid(i,o) merged into one
    2048-elem drain; tanh(c) at 2048-elem super-tiles.
  - Per-level super-tiles are software-pipelined: the f/ht/cagg pass of
    super st is zipped block-by-block with the deferred iou pass of
    super st-1 so the in-order PE/ACT queues never head-block; deep
    levels (m<=512) split into half-column supers to overlap their
    serial chains.
  - Per-tree h sums come from the ht pair-sum tensors (half the reduce
    reads), chunked per super so no big reduce head-blocks the DVE
    queue.

Contract notes vs the reference: the h input is unused (shape only);
c, b_iou, b_in, b_mid, b_out are all-zero per the problem's input spec,
so the kernel drops them (only U_f_b is a live bias).
"""

import os
import sys

import ml_dtypes
import numpy as np

BFNP = ml_dtypes.bfloat16
E4NP = ml_dtypes.float8_e4m3

for _p in ("/opt/trn_rl_repo", "/root/.axon_site/_ro/trn_rl_repo"):
    if os.path.isdir(_p) and _p not in sys.path:
        sys.path.insert(0, _p)

import concourse.bass as bass
import concourse.mybir as mybir
import concourse.tile as tile
from concourse import bacc
from concourse.bass_utils import run_bass_kernel_spmd

P = 128
F32 = mybir.dt.float32
BF16 = mybir.dt.bfloat16
FP8 = mybir.dt.float8e4
H = 256           # hidden size (2 partition chunks)
NB = 32           # trees per core
LEAF = 256        # leaves per tree
COLS = NB * LEAF  # leaf columns per core = 8192
SUB = 512         # iou subtile (psum-coupled)
SUP = 1024        # elementwise/ACT super-tile
AF = mybir.ActivationFunctionType
OP = mybir.AluOpType
DR = mybir.MatmulPerfMode.DoubleRow
QS = 16.0         # weight quant scale (weights x16, ACT scale 1/16)

_PROG = None


def _build_program():
    nc = bacc.Bacc("TRN2", target_bir_lowering=False, debug=False,
                   num_devices=8)

    xT = nc.dram_tensor("xT", [P, 2, COLS], FP8, kind="ExternalInput")
    wiouT = nc.dram_tensor("wiouT", [P, 2, 768], FP8, kind="ExternalInput")
    uiouT = nc.dram_tensor("uiouT", [P, 2, 768], FP8, kind="ExternalInput")
    ufT = nc.dram_tensor("ufT", [P, 2, 256], FP8, kind="ExternalInput")
    ufb = nc.dram_tensor("ufb", [P, 2], F32, kind="ExternalInput")
    ufb8 = nc.dram_tensor("ufb8", [1, 2, P], FP8, kind="ExternalInput")
    ones8 = nc.dram_tensor("ones8", [1, 512], FP8, kind="ExternalInput")
    winT = nc.dram_tensor("winT", [P, 5, P], BF16, kind="ExternalInput")
    emoT = nc.dram_tensor("emoT", [P, NB], BF16, kind="ExternalInput")
    wmidT = nc.dram_tensor("wmidT", [P, 64], F32, kind="ExternalInput")
    woutT = nc.dram_tensor("woutT", [P, 4], F32, kind="ExternalInput")
    out_t = nc.dram_tensor("out_t", [4, NB], F32, kind="ExternalOutput")

    inv = 1.0 / QS

    with tile.TileContext(nc) as tc:
        with (
            tc.tile_pool(name="wp", bufs=1) as wp,
            tc.tile_pool(name="pers", bufs=1) as pers,
        ):
            wiou_sb = wp.tile([P, 2, 768], FP8)
            uiou_sb = wp.tile([P, 2, 768], FP8)
            uf_sb = wp.tile([P, 2, 256], FP8)
            ufb_sb = wp.tile([P, 2], F32)
            ufb8_sb = wp.tile([1, 2, P], FP8)
            ones_sb = wp.tile([1, 512], FP8)
            win_sb = wp.tile([P, 5, P], BF16)
            emo_sb = wp.tile([P, NB], BF16)
            wmid_sb = wp.tile([P, 64], F32)
            wout_sb = wp.tile([P, 4], F32)
            # leaf-critical weights first; the rest are queued later (after
            # the first leaf X tiles) since they are not needed until L7+
            nc.sync.dma_start(wiou_sb[:], wiouT[:])
            late_dmas = [(uf_sb, ufT), (ufb_sb, ufb), (ufb8_sb, ufb8),
                         (ones_sb, ones8), (uiou_sb, uiouT),
                         (win_sb, winT), (emo_sb, emoT), (wmid_sb, wmidT),
                         (wout_sb, woutT)]

            # persistent level storage: h fp8 (levels 8..1), c bf16
            h_lv = {8: pers.tile([P, 2, COLS], FP8, name="h8")}
            c_lv = {8: pers.tile([P, 2, COLS], BF16, name="c8")}
            for d in range(7, 0, -1):
                m = NB * (2 ** d)
                h_lv[d] = pers.tile([P, 2, m], FP8, name=f"h{d}")
                c_lv[d] = pers.tile([P, 2, m], BF16, name=f"c{d}")
            h_lv[0] = pers.tile([P, 2, NB], BF16, name="h0")
            c_lv[0] = pers.tile([P, 2, NB], BF16, name="c0")
            ht_lv = {d: pers.tile([P, 2, NB * (2 ** d)], FP8, name=f"ht{d}")
                     for d in range(8)}
            hsum = pers.tile([P, 2, NB], F32)
            hlast = pers.tile([P, 2, NB], F32)
            nc.vector.memset(hsum[:], 0.0)
            # tiny dummy sigmoid: pulls ACT_TABLE_LOAD + const loads to t=0,
            # overlapping the input DMAs instead of the first leaf drain
            nc.scalar.activation(hlast[:, :, 0:1], hsum[:, :, 0:1],
                                 AF.Sigmoid)

            with (
                tc.tile_pool(name="pps", bufs=1, space="PSUM") as pps,
                tc.tile_pool(name="pa", bufs=2) as pa,
            ):
                def ka(n):
                    """PE keep-alive: dummy DoubleRow weight loads. The HAM
                    clock gate drops the PE to 1.2GHz after ~3.4us of idle;
                    these occupy the PE during dependency stalls with no
                    semantic effect (every real matmul self-loads)."""
                    for _ in range(n):
                        nc.tensor.ldweights(wiou_sb[:, :, 0:P], perf_mode=DR)

                def iou_sub(rhs, w_sb, w, q, io_sb, u_sb, tag):
                    """One 512-col iou subtile: 6 DoubleRow matmuls into
                    [P,4,512]+[P,2,512] PSUM, then merged sigmoid(i,o)
                    (4w elems) and tanh(u) (2w) drains into super-tile
                    SBUF slices."""
                    psio = pps.tile([P, 4, SUB], F32, tag="psIO", bufs=1,
                                    name=f"psio_{tag}")
                    psu = pps.tile([P, 2, SUB], F32, tag="psUF", bufs=2,
                                   name=f"psu_{tag}")
                    for mm in range(6):
                        dst = psio[:, mm, :w] if mm < 4 else \
                            psu[:, mm - 4, :w]
                        if w >= P:
                            nc.tensor.matmul(dst,
                                             w_sb[:, :, mm * P:(mm + 1) * P],
                                             rhs, start=True, stop=True,
                                             perf_mode=DR)
                        else:
                            for k in range(2):
                                nc.tensor.matmul(
                                    dst, w_sb[:, k, mm * P:(mm + 1) * P],
                                    rhs[:, k, :], start=(k == 0),
                                    stop=(k == 1))
                    qs = slice(q * SUB, q * SUB + w)
                    nc.scalar.activation(io_sb[:, :, qs], psio[:, :, :w],
                                         AF.Sigmoid, scale=inv)
                    nc.scalar.activation(u_sb[:, :, qs], psu[:, :, :w],
                                         AF.Tanh, scale=inv)

                def f_block(jb, wj, c0, h_prev, f_sb, tag):
                    """f gates for one 512-children block: 2 DoubleRow
                    matmuls (one per chunk) into [P,2,512] PSUM, 2 sigmoid
                    drains (per-chunk bias)."""
                    psf = pps.tile([P, 2, SUB], F32, tag="psUF", bufs=2,
                                   name=f"psf_{tag}")
                    cs = slice(c0 + jb * 512, c0 + jb * 512 + wj)
                    for g in range(2):
                        if wj >= P:
                            nc.tensor.matmul(
                                psf[:, g, :wj],
                                uf_sb[:, :, g * P:(g + 1) * P],
                                h_prev[:, :, cs], start=True, stop=True,
                                perf_mode=DR)
                        else:
                            for k in range(2):
                                nc.tensor.matmul(
                                    psf[:, g, :wj],
                                    uf_sb[:, k, g * P:(g + 1) * P],
                                    h_prev[:, k, cs],
                                    start=(k == 0), stop=(k == 1))
                    for g in range(2):
                        nc.scalar.activation(
                            f_sb[:, g, jb * 512:jb * 512 + wj],
                            psf[:, g, :wj],
                            AF.Sigmoid, scale=inv, bias=ufb_sb[:, g:g + 1])

                # ---------------- leaves (level 8) ----------------
                h8, c8 = h_lv[8], c_lv[8]
                for st in range(COLS // SUP):
                    ss = slice(st * SUP, (st + 1) * SUP)
                    io_sb = pa.tile([P, 4, SUP], BF16, tag="iob", bufs=3,
                                    name=f"io_L{st}")
                    u_sb = pa.tile([P, 2, SUP], BF16, tag="ub", bufs=3,
                                   name=f"u_L{st}")
                    for half in range(2):
                        xk = pa.tile([P, 2, 512], FP8, tag="xk", bufs=4,
                                     name=f"xk_{st}_{half}")
                        nc.sync.dma_start(
                            xk[:], xT[:, :, st * SUP + half * 512:
                                      st * SUP + half * 512 + 512])
                        iou_sub(xk[:], wiou_sb, SUB, half, io_sb, u_sb,
                                f"L{st}_{half}")
                    if st == 0:
                        for sb, dr_ in late_dmas:
                            nc.sync.dma_start(sb[:], dr_[:])
                    # c = i*u; h = o*tanh(c)
                    nc.vector.tensor_mul(c8[:, :, ss], io_sb[:, 0:2, :],
                                         u_sb[:])
                    t_sb = pa.tile([P, 2, SUP], BF16, tag="tb", bufs=3,
                                   name=f"tl_{st}")
                    nc.scalar.activation(t_sb[:], c8[:, :, ss], AF.Tanh)
                    nc.vector.tensor_mul(h8[:, :, ss], io_sb[:, 2:4, :],
                                         t_sb[:])
                # last leaf (tree-local 255) of each tree
                nc.vector.tensor_copy(hlast[:], h8[:, :, 255::256])

                # ---------------- levels 7..0 ----------------
                # `pending` carries one deferred iou super-pass ACROSS level
                # boundaries: (emit_q(q), emit_tail(), n_q). With
                # sup_d = min(1024, m/2), children(d, 0) reads only the
                # first half of level d+1, which the carried last super
                # never covers, so the carry is always dependency-safe.
                pending = None

                def flush_pending():
                    nonlocal pending
                    if pending is not None:
                        eq, et, nq, qd = pending
                        for q in range(qd, nq):
                            eq(q)
                        et()
                        pending = None

                for d in range(7, -1, -1):
                    m = NB * (2 ** d)
                    h_prev, c_prev = h_lv[d + 1], c_lv[d + 1]
                    h_cur, c_cur = h_lv[d], c_lv[d]
                    ht = ht_lv[d]
                    sup_d = SUP if m > 512 else max(m // 2, 16)
                    n_tree = 2 ** d          # cols per tree at this level
                    deep = m <= 512

                    def iou_q(args, q, d=d, ht=ht, sup_d=sup_d):
                        (sw, st, io_sb, u_sb, cagg, tag) = args
                        w = min(SUB, sw - q * SUB)
                        base = st * sup_d + q * SUB
                        iou_sub(ht[:, :, base:base + w], uiou_sb, w, q,
                                io_sb, u_sb, f"{tag}_q{q}")

                    def iou_tail(args, d=d, ht=ht, sup_d=sup_d,
                                 n_tree=n_tree, h_cur=h_cur, c_cur=c_cur):
                        (sw, st, io_sb, u_sb, cagg, tag) = args
                        ssl = slice(st * sup_d, st * sup_d + sw)
                        iub = pa.tile([P, 2, SUP], BF16, tag="iub", bufs=2,
                                      name=f"iu_{tag}")
                        nc.vector.tensor_mul(iub[:, :, :sw],
                                             io_sb[:, 0:2, :sw],
                                             u_sb[:, :, :sw])
                        nc.vector.tensor_add(c_cur[:, :, ssl],
                                             iub[:, :, :sw],
                                             cagg[:, :, :sw])
                        t_sb = pa.tile([P, 2, SUP], BF16, tag="tb", bufs=3,
                                       name=f"t_{tag}")
                        nc.scalar.activation(t_sb[:, :, :sw],
                                             c_cur[:, :, ssl], AF.Tanh)
                        nc.vector.tensor_mul(h_cur[:, :, ssl],
                                             io_sb[:, 2:4, :sw],
                                             t_sb[:, :, :sw])
                        # hsum contribution of this super's trees via ht
                        t0 = (st * sup_d) // n_tree
                        tn = sw // n_tree if n_tree <= sw else 1
                        part = pa.tile([P, 2, NB], F32, tag="part", bufs=2,
                                       name=f"part_{tag}")
                        if n_tree > 1:
                            nc.vector.tensor_reduce(
                                part[:, :, :tn],
                                ht[:, :, ssl].rearrange(
                                    "p k (t n) -> p k t n", t=tn),
                                axis=mybir.AxisListType.X, op=OP.add)
                        else:
                            nc.vector.tensor_copy(part[:, :, :tn],
                                                  ht[:, :, ssl])
                        nc.gpsimd.tensor_add(hsum[:, :, t0:t0 + tn],
                                             part[:, :, :tn],
                                             hsum[:, :, t0:t0 + tn])

                    def emit_children(st, sw, cagg, f_sb,
                                      d=d, ht=ht, sup_d=sup_d, deep=deep,
                                      h_prev=h_prev, c_prev=c_prev):
                        """f gates + fc + pair sums for super-tile st,
                        zipped block-by-block with the pending deferred iou
                        pass (keeps the in-order PE queue fed)."""
                        nonlocal pending
                        c0 = 2 * st * sup_d
                        cw = 2 * sw
                        # ht pair sums (iou input) first: they depend only
                        # on h_prev, so they must not queue behind cagg.
                        # bit-reversed column order makes the children pair
                        # (2j, 2j+1) the contiguous halves of each tree's
                        # children block.
                        nd = 2 ** d
                        for k in range(2):
                            hv = h_prev[:, k, c0:c0 + cw].rearrange(
                                "p (t two nd) -> p t two nd", two=2, nd=nd)
                            nc.vector.tensor_add(
                                ht[:, k, st * sup_d:st * sup_d + sw]
                                .rearrange("p (t nd) -> p t nd", nd=nd),
                                hv[:, :, 0, :], hv[:, :, 1, :])
                        nblk = (cw + 511) // 512
                        for jb in range(nblk):
                            wj = min(512, cw - jb * 512)
                            f_block(jb, wj, c0, h_prev, f_sb,
                                    f"B{d}_{st}_{jb}")
                            if pending is not None:
                                eq, et, nq, qd = pending
                                want = min(nq, (jb + 1) * nq // nblk)
                                while qd < want:
                                    eq(qd)
                                    qd += 1
                                pending = (eq, et, nq, qd)
                        csl = slice(c0, c0 + cw)
                        nc.vector.tensor_mul(c_prev[:, :, csl],
                                             f_sb[:, :, :cw],
                                             c_prev[:, :, csl])
                        eng = nc.vector
                        for k in range(2):
                            fv = c_prev[:, k, c0:c0 + cw].rearrange(
                                "p (t two nd) -> p t two nd", two=2, nd=nd)
                            eng.tensor_add(
                                cagg[:, k, :sw].rearrange(
                                    "p (t nd) -> p t nd", nd=nd),
                                fv[:, :, 0, :], fv[:, :, 1, :])
                        flush_pending()

                    for st in range((m + sup_d - 1) // sup_d):
                        sw = min(sup_d, m - st * sup_d)
                        io_sb = pa.tile([P, 4, SUP], BF16, tag="iob", bufs=3,
                                        name=f"io_{d}_{st}")
                        u_sb = pa.tile([P, 2, SUP], BF16, tag="ub", bufs=3,
                                       name=f"u_{d}_{st}")
                        cagg = pa.tile([P, 2, SUP], BF16, tag="cagg", bufs=2,
                                       name=f"cagg_{d}_{st}")
                        f_sb = pa.tile([P, 2, 2 * SUP], BF16, tag="fb",
                                       bufs=2, name=f"f_{d}_{st}")
                        emit_children(st, sw, cagg, f_sb)
                        args = (sw, st, io_sb, u_sb, cagg, f"B{d}_{st}")
                        nqs = (sw + SUB - 1) // SUB
                        pending = (
                            lambda q, a=args, f=iou_q: f(a, q),
                            lambda a=args, f=iou_tail: f(a),
                            nqs, 0)
                    flush_pending()

                # ---------------- head (fp32 tail) ----------------
                h_root = h_lv[0]
                inner = pa.tile([P, 2, NB], BF16)
                nc.vector.tensor_sub(inner[:], hsum[:], hlast[:])
                nc.vector.tensor_scalar_mul(inner[:], inner[:], 1.0 / 509.0)
                y2_sb = pa.tile([P, NB], F32)
                nc.vector.memset(y2_sb[:], 0.0)

                py1 = pps.tile([P, NB], F32, tag="psUF", bufs=2, name="py1")
                chunks = [h_root[:, 0, :], h_root[:, 1, :],
                          inner[:, 0, :], inner[:, 1, :], emo_sb[:]]
                for k in range(5):
                    nc.tensor.matmul(py1[:], win_sb[:, k, :], chunks[k],
                                     start=(k == 0), stop=(k == 4))
                y1_sb = pa.tile([P, NB], F32)
                nc.scalar.activation(y1_sb[:], py1[:], AF.Relu)
                py2 = pps.tile([64, NB], F32, tag="psUF", bufs=2, name="py2")
                nc.tensor.matmul(py2[:], wmid_sb[:], y1_sb[:])
                nc.scalar.activation(y2_sb[:64, :], py2[:], AF.Relu)
                po = pps.tile([4, NB], F32, tag="psUF", bufs=2, name="po")
                nc.tensor.matmul(po[:], wout_sb[:], y2_sb[:])
                o_sb = pa.tile([4, NB], F32)
                nc.scalar.activation(o_sb[:], po[:], AF.Sigmoid)
                nc.sync.dma_start(out_t[:], o_sb[:])

    nc.finalize()
    return nc


def _chunked(w):
    """[K, M] host array -> [P, K//P, M] device layout (K on partitions)."""
    k, m = w.shape
    return np.ascontiguousarray(w.reshape(k // P, P, m).transpose(1, 0, 2))


def _prep_shared(W_iou, U_iou, b_iou, U_f_w, U_f_b, W_in, b_in, W_mid, b_mid,
                 W_out, b_out):
    f = np.float32
    wiouT = _chunked(np.ascontiguousarray(W_iou.T).astype(f) * QS).astype(E4NP)
    uiouT = _chunked(np.ascontiguousarray(U_iou.T).astype(f) * QS).astype(E4NP)
    ufT = _chunked(np.ascontiguousarray(U_f_w.T).astype(f) * QS).astype(E4NP)
    ufb_h = np.ascontiguousarray(U_f_b.reshape(2, P).T).astype(f)
    ufb8_h = (U_f_b.reshape(1, 2, P).astype(f) * QS).astype(E4NP)
    ones_h = np.ones((1, 512), f).astype(E4NP)
    winT = np.zeros((640, P), f)
    winT[:544] = W_in.T
    winT = _chunked(winT).astype(BFNP)
    wmidT = np.ascontiguousarray(W_mid.T).astype(f)
    woutT = np.zeros((P, 4), f)
    woutT[:64] = W_out.T
    return dict(wiouT=wiouT, uiouT=uiouT, ufT=ufT, ufb=ufb_h,
                ufb8=ufb8_h, ones8=ones_h,
                winT=winT, wmidT=wmidT, woutT=woutT)


def _run(X, emo, shared, trace=False):
    global _PROG
    if _PROG is None:
        _PROG = _build_program()
    nc = _PROG

    # per-tree bit-reversed leaf order: makes every level's child pairs
    # the contiguous halves of each tree's block on device
    br = np.zeros(LEAF, np.int64)
    for i in range(LEAF):
        br[i] = int(format(i, "08b")[::-1], 2)

    in_maps = []
    for cc in range(8):
        Xc = X[cc * NB:(cc + 1) * NB, 255:511, :][:, br, :]
        xT = Xc.transpose(2, 0, 1).reshape(256, COLS)
        xT = np.ascontiguousarray(
            xT.reshape(2, P, COLS).transpose(1, 0, 2)).astype(E4NP)
        emoT = np.zeros((P, NB), BFNP)
        emoT[:32] = emo[cc * NB:(cc + 1) * NB].T.astype(BFNP)
        in_maps.append(dict(xT=xT, emoT=emoT, **shared))

    res = None
    for attempt in range(3):
        try:
            res = run_bass_kernel_spmd(nc, in_maps, core_ids=list(range(8)),
                                       trace=trace)
            break
        except Exception:
            if attempt == 2:
                raise
    out = np.concatenate([res.results[cc]["out_t"].T for cc in range(8)],
                         axis=0)
    return np.ascontiguousarray(out.astype(np.float32)), res


def kernel(X, h, c, emo, W_iou, U_iou, b_iou, U_f_w, U_f_b,
           W_in, b_in, W_mid, b_mid, W_out, b_out, **kwargs):
    X = np.asarray(X, np.float32)
    emo = np.asarray(emo, np.float32)
    shared = _prep_shared(np.asarray(W_iou), np.asarray(U_iou),
                          np.asarray(b_iou), np.asarray(U_f_w),
                          np.asarray(U_f_b), np.asarray(W_in),
                          np.asarray(b_in), np.asarray(W_mid),
                          np.asarray(b_mid), np.asarray(W_out),
                          np.asarray(b_out))
    out, _ = _run(X, emo, shared)
    return out
